# revision 1
# baseline (speedup 1.0000x reference)
"""Trainium2 Bass kernel for NonLocalAttention (fused 1x1 convs + spatial softmax attention).

Reference computation (N=2, C=64, FC=64, CR=32, H=W=96, HW=9216):
    q = relu(wq @ x + bq)          [N, 32, HW]
    k = relu(wk @ fm + bk)         [N, 32, HW]
    v = relu(wa @ fm + ba)         [N, 64, HW]
    s = softmax(q^T k, axis=keys)  [N, HW, HW]
    o = s @ v^T                    [N, HW, 64]
    out = relu(wo @ [x; o^T] + bo) [N, 64, HW]

Sharding: 8 cores = batch(2) x query-rows(4).  Each core handles 2304 query
pixels of one batch element and needs the full fusionmap of that batch.

Per-core kernel (flash-style, score never goes to HBM):
  - score is computed TRANSPOSED: st[key, q] = k^T q via row-packed (K=32)
    matmuls, 3 key-tiles of 128 at a time into 3 PSUM banks.
  - exp ALTERNATES whole steps between ScalarE (exact LUT exp -> bf16) and
    VectorE (bf16 Schraudolph: st16 = int16(s*128*log2e + B); the int16 bit
    pattern read as bf16 is 2^x with linear mantissa interp, a ~3% sawtooth
    that softmax normalization largely cancels).  Whole-step alternation
    matters: the tile scheduler serializes cross-engine readers of a shared
    tile, so bank-level splits ran lockstep instead of concurrently.
    Scores are >= 0 (q,k relu'd) and <= ~6.6, so no max subtraction.
  - mm2 payloads lag two steps behind mm1 so the in-order PE queue never
    sits behind an exp that has not finished yet.
  - a ~4us junk-matmul warm-up during the initial DMA wait locks the HAM
    clock gate into full rate: without it, runs that enter the loop cold
    LOCK at half PE clock (~215us vs ~180us, bimodal).
  - second matmul contracts keys with lhsT = [v^T | 1] so PSUM row 64
    accumulates the softmax denominator for free.
  - normalize: denominator row spread to [64, W] via DMA, DVE reciprocal
    (parallel lanes, ~100x faster than on 1 partition), gathered back,
    broadcast by a K=1 matmul; then the output 1x1 conv (wo), relu, DMA out.
"""

import sys

sys.path.insert(0, "/opt/trn_rl_repo")

from contextlib import ExitStack

import ml_dtypes
import numpy as np

import concourse.bacc as bacc
import concourse.bass as bass
import concourse.tile as tile
from concourse import mybir
from concourse import bass_utils

C = 64
FC = 64
CR = 32
N = 2
H = W = 96
HW = H * W            # 9216
NCORES = 8
QPC = HW // 4         # queries per core = 2304
NKT = HW // 128       # 72 key tiles
G = 3                 # row-packing group (3 key tiles concurrently)
NJ = NKT // G         # 24 key-tile groups
NT_MACRO = 4 * NJ + NJ // 2   # 108 macro-steps (four 512 chunks + 256 pairs)
# the 256-query chunk sits MID-KERNEL: first chunk needs the full 512 width
# to drive the HAM clock gate warm through the conv phase, and a trailing
# small chunk left the tail at half clock.
QCHUNKS = [(0, 512), (512, 512), (1024, 256), (1280, 512), (1792, 512)]

F32 = mybir.dt.float32
F32R = mybir.dt.float32r
BF16 = mybir.dt.bfloat16
I16 = mybir.dt.int16
ATT = BF16

# bf16 Schraudolph constants: y = int16(x * 128*log2e + B16C); the int16
# bit pattern read as bf16 is ~exp(x) with a +-~3% sawtooth (bias is
# HW-calibrated for the DVE's fp32->int16 conversion rounding).
LOG2E = 1.4426950408889634
S16 = 128.0 * LOG2E
B16C = 16248.6


def scalar_step(qi, mi, t):
    """True if ScalarE exp's this whole step (VectorE Schraudolphs the
    others).  Whole-step alternation: a score tile is only ever read by ONE
    engine -- the tile scheduler serializes cross-engine readers of a shared
    tile, so bank-level splits ran lockstep instead of concurrently.
    Chunk 0 is Scalar-heavy because the DVE is busy with conv relus there.
    The finalize-copy steps (mi 4 and 6 carry the DS/rbS ScalarE copies) are
    forced to DVE so ScalarE never stacks exp+copy in one step; mi 5 and 7
    are forced to ScalarE to keep the 12/12 per-chunk balance."""
    if qi == 0:
        return t % 4 != 3
    if mi in (4, 6):
        return False
    if mi in (5, 7):
        return True
    return t % 2 == 0


def build_bass():
    nc = bacc.Bacc(
        "TRN2", target_bir_lowering=False, debug=False, num_devices=NCORES
    )

    x_aug = nc.dram_tensor("x_aug", [C + 1, QPC], F32R, kind="ExternalInput")
    x_bf = nc.dram_tensor("x_bf", [C + 1, QPC], BF16, kind="ExternalInput")
    fm_aug = nc.dram_tensor("fm_aug", [FC + 1, HW], BF16, kind="ExternalInput")
    wq_aug = nc.dram_tensor("wq_aug", [C + 1, CR], BF16, kind="ExternalInput")
    wk_aug = nc.dram_tensor("wk_aug", [FC + 1, CR], BF16, kind="ExternalInput")
    wa_aug = nc.dram_tensor("wa_aug", [FC + 1, 128], BF16, kind="ExternalInput")
    wox_aug = nc.dram_tensor("wox_aug", [C + 1, C], F32R, kind="ExternalInput")
    woa_t = nc.dram_tensor("woa_t", [C, C], F32R, kind="ExternalInput")
    out_d = nc.dram_tensor("out_c", [C, QPC], F32, kind="ExternalOutput")

    with tile.TileContext(nc) as tc, ExitStack() as ctx:
        consts = ctx.enter_context(tc.tile_pool(name="consts", bufs=1))
        stp = ctx.enter_context(tc.tile_pool(name="stp", bufs=3))
        wk_pool = ctx.enter_context(tc.tile_pool(name="work", bufs=3))
        # PSUM: 2x3 banks score tiles + 2x1 bank shared acc/fin/v-conv = 8
        psA = ctx.enter_context(tc.tile_pool(name="psA", bufs=2, space="PSUM"))
        psO = ctx.enter_context(tc.tile_pool(name="psO", bufs=2, space="PSUM"))

        # ---- constants / inputs in SBUF ----
        NQT = 4
        HWQ = HW // NQT  # 2304 = 18 key tiles per quarter
        FMq = [
            consts.tile([FC + 1, HWQ], BF16, tag=f"fm{p}", name=f"FM{p}")
            for p in range(NQT)
        ]
        XA = consts.tile([C + 1, QPC], F32R)         # x chunk + ones row
        XB = consts.tile([C + 1, QPC], BF16)         # bf16 copy for q-conv
        WQ = consts.tile([C + 1, CR], BF16)
        WK = consts.tile([FC + 1, CR], BF16)
        WA = consts.tile([FC + 1, 128], BF16)
        WOX = consts.tile([C + 1, C], F32R)
        WOA = consts.tile([C, C], F32R)
        # DMA order = critical path order: k-conv q0 needs WK+FM0, q-conv
        # chunk 0 needs WQ + the first 512 columns of XA; big transfers are
        # split so the first consumers start sooner.
        nc.sync.dma_start(WK[:], wk_aug.ap())
        nc.sync.dma_start(WQ[:], wq_aug.ap())
        nc.sync.dma_start(FMq[0][:, 0:1536], fm_aug.ap()[:, 0:1536])
        nc.sync.dma_start(XB[:, 0:512], x_bf.ap()[:, 0:512])
        nc.sync.dma_start(FMq[0][:, 1536:HWQ], fm_aug.ap()[:, 1536:HWQ])
        nc.sync.dma_start(WA[:], wa_aug.ap())
        nc.sync.dma_start(XB[:, 512:QPC], x_bf.ap()[:, 512:QPC])
        for p in range(1, NQT):
            nc.sync.dma_start(FMq[p][:], fm_aug.ap()[:, p * HWQ : (p + 1) * HWQ])
        nc.sync.dma_start(XA[:], x_aug.ap())
        nc.sync.dma_start(WOX[:], wox_aug.ap())
        nc.sync.dma_start(WOA[:], woa_t.ap())

        def fm_kt(kt):  # [65, 128] slice of fusionmap for key tile kt
            p, i = divmod(kt, 18)
            return FMq[p][:, 128 * i : 128 * (i + 1)]

        # KR: k channels row-packed: partitions 32g..32g+31 hold key tile
        # kt=3j+g at free block j; per-quarter for earlier start.
        # QR: per-chunk tiles, q replicated on partition groups 0..2.
        KRq = [
            consts.tile([128, NJ // NQT, 128], ATT, tag=f"kr{p}", name=f"KR{p}")
            for p in range(NQT)
        ]
        QRc = [
            consts.tile([128, qn], ATT, tag=f"qr{ci}", name=f"QR{ci}")
            for ci, (q0, qn) in enumerate(QCHUNKS)
        ]
        # VT: [keys(128), kt, 128]; column C is 1.0 straight out of the
        # v-conv (wa is augmented with a ones column), so mm2 accumulates the
        # softmax denominator in PSUM row C for free.  Columns C+1..127 are
        # zero padding: a 128-wide stationary operand enables the PE's fast
        # weight load for mm2, and the extra acc rows just accumulate zeros.
        NVR = NKT // 4
        VTr = [
            consts.tile([128, 4, 128], ATT, tag=f"vt{r}", name=f"VT{r}")
            for r in range(NVR)
        ]
        ones1 = consts.tile([1, C], F32R)
        nc.gpsimd.memset(ones1[:].bitcast(F32), 1.0)

        # PE warm-up: ~4us of back-to-back junk matmuls while the input DMAs
        # land (the PE is idle until ~12us regardless).  The HAM clock gate
        # is metastable at our steady-state duty cycle: runs that enter the
        # loop cold LOCK at half clock (~215us vs ~180us).  Warming during
        # the DMA window guarantees the fast mode, for free.
        warm_rhs = consts.tile([1, 512], F32R)
        nc.gpsimd.memset(warm_rhs[:].bitcast(F32), 1.0)
        warm_ps = psA.tile([128, G, 512], F32, tag="sc", name="warm")
        # 12 x ~620ns = 7.4us of continuous PE busy: >= 2 HAM windows, so a
        # full busy window is covered at ANY phase of the free-running gate
        # (10 mms = 6.2us left an ~18% phase gap; cold-lock clusters matched
        # correlated boot phases of back-to-back runs).
        for _ in range(12):
            nc.tensor.matmul(warm_ps[0:C, 0, :], ones1[:], warm_rhs[:])

        # Preload the exp table set (~2.7us) off the critical path, before
        # the first real exp.
        dummy = wk_pool.tile([1, 1], F32, tag="dummy", name="dummy")
        nc.scalar.activation(
            dummy[:], ones1[0:1, 0:1].bitcast(F32),
            mybir.ActivationFunctionType.Exp,
        )

        # ---- phase 1: q / k convs ----
        # Column-tiled on the PE: the three 32-wide groups (tile_position
        # col_grp) run concurrently and write the row-packed KR layout
        # directly -- group g computes key tiles 3j+g, so no interleave DMA.
        def k_quarter(p, j0, jn):
            fmv = FMq[p].rearrange("p (j g c) -> p j g c", g=G, c=128)
            ps = psA.tile([128, G, 512], F32, tag="sc", name="kps")
            for g in range(G):
                nc.tensor.matmul(
                    ps[32 * g : 32 * g + 32, 0, 0 : jn * 128],
                    WK[:],
                    fmv[:, j0 : j0 + jn, g, :],
                    tile_position=(0, 32 * g),
                )
            nc.vector.tensor_scalar_max(
                KRq[p][0:96, j0 : j0 + jn, :], ps[0:96, 0, 0 : jn * 128], 0.0
            )

        # q chunk ci: same trick with a shared rhs replicates q into the
        # three partition groups without any DMA.
        def q_chunk(ci):
            q0, qn = QCHUNKS[ci]
            ps = psA.tile([128, G, 512], F32, tag="sc", name="qps")
            for g in range(G):
                nc.tensor.matmul(
                    ps[32 * g : 32 * g + 32, 0, 0:qn],
                    WQ[:],
                    XB[:, q0 : q0 + qn],
                    tile_position=(0, 32 * g),
                )
            nc.vector.tensor_scalar_max(
                QRc[ci][0:96, 0:qn], ps[0:96, 0, 0:qn], 0.0
            )

        # prologue: only what the first attention step needs (quarter 0 of
        # the k conv and query chunk 0); the rest is emitted inside chunk
        # 0's loop so the PE, DVE and ScalarE all saturate from the start.
        k_quarter(0, 0, 4)
        q_chunk(0)
        k_quarter(0, 4, 2)

        # v^T conv round r: out[key, 0:64] = relu'd v, out[key, 64] = 1.0,
        # out[key, 65:128] = 0 (wa padding)
        def v_round(r):
            ps = psO.tile([128, 512], F32, tag="acc")
            for i in range(4):
                kt = 4 * r + i
                nc.tensor.matmul(
                    ps[:, 128 * i : 128 * (i + 1)], fm_kt(kt), WA[:]
                )
            nc.vector.tensor_scalar_max(VTr[r][:], ps[:], 0.0)

        # ---- phase 2: attention + output conv, per query chunk ----
        # The normalize + output conv of chunk qc is emitted inside chunk
        # qc+1's loop, in two halves: the reciprocal chain early (j==1) so
        # its DMA/DVE latency is long gone when the PE-side half (j==4)
        # reaches the in-order PE queue (a stalled PE > 3.4us re-throttles
        # the HAM clock gate to half rate).
        def finalize_a(acc, q0, qn, off=0):
            # row C of acc = sum_k exp(score).  Spread the [1, qn] row over
            # 64 partitions by DMA so the DVE reciprocal runs on parallel
            # lanes (~100x faster than on 1 partition), gather back.
            Wd = qn // 64
            DS = wk_pool.tile([1, 512], F32, tag="ds", name="ds")
            nc.scalar.copy(DS[:, 0:qn], acc[C : C + 1, off : off + qn])
            DD = wk_pool.tile([64, 8], F32, tag="dd", name="dd")
            nc.scalar.dma_start(DD[:, 0:Wd], DS[:, 0:qn])
            RR = wk_pool.tile([64, 8], F32, tag="rr", name="rr")
            with nc.allow_low_precision(reason="softmax denom reciprocal"):
                nc.vector.reciprocal(RR[:, 0:Wd], DD[:, 0:Wd])
            RCP = wk_pool.tile([1, 512], F32R, tag="rc", name="rc")
            nc.scalar.dma_start(RCP[:, 0:qn].bitcast(F32), RR[:, 0:Wd])
            return RCP

        def finalize_b(acc, q0, qn, RCP, off=0):
            # Broadcast 1/denom to 64 partitions with a K=1 matmul.
            rb_ps = psA.tile([128, G, 512], F32, tag="sc", name="rb_ps")
            nc.tensor.matmul(rb_ps[0:C, 0, 0:qn], ones1[:], RCP[:, 0:qn])
            rbS = wk_pool.tile([C, 512], F32, tag="rbS", name="rbS")
            nc.scalar.copy(rbS[:, 0:qn], rb_ps[0:C, 0, 0:qn])
            return rbS

        def finalize_c(acc, q0, qn, rbS, off=0):
            attn = wk_pool.tile([C, 512], F32R, tag="attn", name="attn")
            nc.vector.tensor_mul(attn[:, 0:qn], acc[0:C, off : off + qn], rbS[:, 0:qn])
            # out = relu(wo_x @ x + wo_a @ attn + bo); fin is allocated only
            # after the attn mul (acc's last reader) so the psO slot reuse
            # of acc(qi-1) cannot cycle.
            fin = psO.tile([128, 512], F32, tag="acc", name="fin")
            nc.tensor.matmul(
                fin[0:C, 0:qn], WOX[:], XA[:, q0 : q0 + qn],
                start=True, stop=False,
            )
            nc.tensor.matmul(
                fin[0:C, 0:qn], WOA[:], attn[:, 0:qn],
                start=False, stop=True,
            )
            outs = wk_pool.tile([C, 512], F32, tag="outs", name="outs")
            nc.vector.tensor_scalar_max(outs[:, 0:qn], fin[0:C, 0:qn], 0.0)
            nc.sync.dma_start(out_d.ap()[:, q0 : q0 + qn], outs[:, 0:qn])

        # Flat software-pipelined emission over macro-steps.  A macro-step
        # is one [128, G, 512] score tile: one j-group for 512-wide chunks,
        # TWO j-groups side by side for the 256-wide chunk -- every step has
        # identical engine load (1536 exp elems, 4*512 PE stream cycles), so
        # the PE never drops to the HAM half-clock regime mid-kernel.
        # mm2 payloads lag two macro-steps: the PE queue never sits behind
        # an exp that hasn't finished yet.
        MM2_LAG = 2
        macro = []  # (qi, subs) with subs = [j] or [j0, j1]
        for qi, (q0, qn) in enumerate(QCHUNKS):
            if qn == 512:
                macro += [(qi, [j]) for j in range(NJ)]
            else:
                macro += [(qi, [2 * u, 2 * u + 1]) for u in range(NJ // 2)]
        accs = [None] * len(QCHUNKS)
        pend_mm2 = []
        pend_fin = None
        pend_rcp = None
        vr_next = 0

        def emit_mm2(qi, subs, srcs):
            q0, qn = QCHUNKS[qi]
            for (kt0, st, f0) in srcs:
                for g in range(G):
                    kt = kt0 + g
                    nc.tensor.matmul(
                        accs[qi][:, 0:qn],
                        VTr[kt // 4][:, kt % 4, :],
                        st[:, g, f0 : f0 + qn],
                        start=(kt == 0),
                        stop=(kt == NKT - 1),
                    )

        for t, (qi, subs) in enumerate(macro):
            q0, qn = QCHUNKS[qi]
            j = subs[0]
            if j == 0:
                accs[qi] = psO.tile([128, 512], F32, tag="acc", name="acc")
                mi = 0
            mi += 1
            if qi == 0:
                # remaining conv work, front-loaded so each k quarter lands
                # well before its first mm1 consumer (quarter p needed from
                # j = 6p) and each q chunk before its chunk starts
                KSCHED = {
                    0: lambda: k_quarter(1, 0, 4),
                    1: lambda: k_quarter(1, 4, 2),
                    3: lambda: k_quarter(2, 0, 4),
                    4: lambda: k_quarter(2, 4, 2),
                    6: lambda: q_chunk(1),
                    8: lambda: k_quarter(3, 0, 4),
                    9: lambda: k_quarter(3, 4, 2),
                }
                if j in KSCHED:
                    KSCHED[j]()
            # later q-convs ride the steady chunks' PE slack (chunk ci
            # needs QRc[ci] only when chunk ci starts)
            elif qi < len(QCHUNKS) - 1 and j == 4:
                q_chunk(qi + 1)
            # keep v-conv a little ahead of the mm2 consumer on chunk 0
            while vr_next < NVR and (qi > 0 or 4 * vr_next <= 3 * j + 10):
                v_round(vr_next)
                vr_next += 1
            sc = psA.tile([128, G, 512], F32, tag="sc")
            kts = []  # (first kt, free offset) per sub-step
            for h, sj in enumerate(subs):
                jq, jj = divmod(sj, NJ // NQT)
                for g in range(G):
                    nc.tensor.matmul(
                        sc[:, g, qn * h : qn * h + qn],
                        KRq[jq][32 * g : 32 * g + 32, jj, :],
                        QRc[qi][32 * g : 32 * g + 32, 0:qn],
                    )
                kts.append((G * sj, qn * h))
            if scalar_step(qi, mi, t):
                st = stp.tile([128, G, 512], ATT, tag="st")
                nc.scalar.activation(
                    st[:], sc[:], mybir.ActivationFunctionType.Exp
                )
            else:
                st_v = stp.tile([128, G, 512], I16, tag="stv")
                nc.vector.tensor_scalar(
                    st_v[:], sc[:], S16, B16C,
                    mybir.AluOpType.mult, mybir.AluOpType.add,
                )
                st = st_v.bitcast(ATT)
            srcs = [(kt0, st, f0) for (kt0, f0) in kts]
            pend_mm2.append((qi, subs, srcs))
            # pair-emission for 512 chunks: two steps' mm1 groups go back to
            # back (their row-group LDWs overlap each other's streams, ~130ns
            # spacing measured), then both trios of mm2s -- halves the count
            # of expensive mm1<->mm2 array transitions.
            if qn == 512 and t % 2 == 1:
                while len(pend_mm2) > MM2_LAG:
                    emit_mm2(*pend_mm2.pop(0))
            elif qn != 512 and len(pend_mm2) > MM2_LAG:
                emit_mm2(*pend_mm2.pop(0))
            if pend_fin is not None:
                if mi == 4:
                    pend_rcp = finalize_a(*pend_fin)
                elif mi == 6:
                    pend_rcp = finalize_b(*pend_fin, pend_rcp)
                elif mi == 7:
                    finalize_c(*pend_fin, pend_rcp)
                    pend_fin = None
                    pend_rcp = None
            if subs[-1] == NJ - 1:
                pend_fin = (accs[qi], q0, qn)
        # tail: the WOX half of the output conv only needs XA, so it issues
        # right behind the mm2 flush while the reciprocal chain runs.
        for p in pend_mm2:
            emit_mm2(*p)
        acc, q0, qn = pend_fin
        rcp = finalize_a(*pend_fin)
        fin = psO.tile([128, 512], F32, tag="acc", name="fin")
        nc.tensor.matmul(
            fin[0:C, 0:qn], WOX[:], XA[:, q0 : q0 + qn],
            start=True, stop=False,
        )
        rbs = finalize_b(*pend_fin, rcp)
        attn = wk_pool.tile([C, 512], F32R, tag="attn", name="attn")
        nc.vector.tensor_mul(attn[:, 0:qn], acc[0:C, 0:qn], rbs[:, 0:qn])
        nc.tensor.matmul(
            fin[0:C, 0:qn], WOA[:], attn[:, 0:qn],
            start=False, stop=True,
        )
        outs = wk_pool.tile([C, 512], F32, tag="outs", name="outs")
        nc.vector.tensor_scalar_max(outs[:, 0:qn], fin[0:C, 0:qn], 0.0)
        nc.sync.dma_start(out_d.ap()[:, q0 : q0 + qn], outs[:, 0:qn])

    nc.compile()
    return nc


_NC_CACHE = None


def _get_nc():
    global _NC_CACHE
    if _NC_CACHE is None:
        _NC_CACHE = build_bass()
    return _NC_CACHE


def make_in_maps(x, fusionmap, wq, bq, wk, bk, wa, ba, wo, bo):
    x = np.asarray(x, np.float32)
    fm = np.asarray(fusionmap, np.float32)
    xf = x.reshape(N, C, HW)
    fmf = fm.reshape(N, FC, HW)
    ones_hw = np.ones((1, HW), np.float32)
    wq_aug = np.concatenate(
        [np.asarray(wq).T, np.asarray(bq)[None, :]], 0
    ).astype(ml_dtypes.bfloat16)
    wk_aug = np.concatenate(
        [np.asarray(wk).T, np.asarray(bk)[None, :]], 0
    ).astype(ml_dtypes.bfloat16)
    # [wa^T | 0 | 0; ba | 1 | 0]: column C evaluates to exactly 1.0 after
    # the conv (ones row of fm_aug x ones), giving mm2 its denominator row;
    # columns C+1..127 are zero padding for the PE fast weight load.
    wa_blk = np.concatenate([np.asarray(wa).T, np.asarray(ba)[None, :]], 0)
    ones_blk = np.concatenate(
        [np.zeros((FC, 1), np.float32), np.ones((1, 1), np.float32)], 0
    )
    pad_blk = np.zeros((FC + 1, 128 - C - 1), np.float32)
    wa_aug = np.concatenate([wa_blk, ones_blk, pad_blk], 1).astype(
        ml_dtypes.bfloat16
    )
    wo = np.asarray(wo, np.float32)
    wox_aug = np.concatenate(
        [wo[:, :C].T, np.asarray(bo)[None, :]], 0
    ).astype(np.float32)
    woa_t = np.ascontiguousarray(wo[:, C:].T).astype(np.float32)

    in_maps = []
    for core in range(NCORES):
        n, c = divmod(core, 4)
        x_chunk = xf[n][:, c * QPC : (c + 1) * QPC]
        x_aug = np.concatenate([x_chunk, ones_hw[:, :QPC]], 0)
        fm_aug = np.concatenate([fmf[n], ones_hw], 0).astype(ml_dtypes.bfloat16)
        in_maps.append(
            {
                "x_aug": np.ascontiguousarray(x_aug),
                "x_bf": np.ascontiguousarray(x_aug.astype(ml_dtypes.bfloat16)),
                "fm_aug": np.ascontiguousarray(fm_aug),
                "wq_aug": wq_aug,
                "wk_aug": wk_aug,
                "wa_aug": wa_aug,
                "wox_aug": wox_aug,
                "woa_t": woa_t,
            }
        )
    return in_maps


def run(in_maps, trace=False, tmpdir=None):
    nc = _get_nc()
    return bass_utils.run_bass_kernel_spmd(
        nc,
        in_maps,
        core_ids=list(range(NCORES)),
        trace=trace,
        tmpdir=tmpdir,
    )


def kernel(**inputs):
    in_maps = make_in_maps(**inputs)
    res = run(in_maps)
    out = np.empty((N, C, HW), np.float32)
    for core in range(NCORES):
        n, c = divmod(core, 4)
        out[n][:, c * QPC : (c + 1) * QPC] = res.results[core]["out_c"]
    return out.reshape(N, C, H, W)


if __name__ == "__main__":
    import reference

    inputs = {k: np.asarray(v) for k, v in reference.setup_inputs().items()}
    got = kernel(**inputs)
    print("kernel output", got.shape, got.dtype)

